# revision 15
# baseline (speedup 1.0000x reference)
"""ComENet-style GNN block on 8 Trainium2 NeuronCores (Bass/Tile SPMD).

Sharding: nodes/edges by graph (contiguous graphs per core, cut to balance
edge counts). Edges assigned to the core owning their TARGET node and sorted
by local target, so scatter stays on-device via one-hot matmuls. Source-node
features are host-gathered per edge (pure input rearrangement); all model
arithmetic runs on device. Weights replicated across cores.

Per-core device program (all matmuls fp32r = full-precision fp32 at bf16 rate):
  1. fold W2@W1 for both edge-feature MLPs (device matmul)
  2. x_local = swish(x @ lin_w.T + b)  (H-major)
  3. per branch: msgs[e] = (feat[e] @ Wc.T) * swish(x_src[e] @ lin_w.T + b)
     (edge-major), scatter = one-hot S matmuls over register-indexed dynamic
     message windows -> agg (node-major) -> PE-transpose -> H-major
  4. conv + lin1/lin2, lincat + residual, 3 residual lins (H-major)
  5. GraphNorm via per-graph moment matmuls (exact: var = E[h^2]-2a m^2+a^2 m^2)
  6. final linear -> out^T, host reassembles [N, H]
"""

import os

os.environ.setdefault("MYCRO_LOCAL_CACHE", "1")

import numpy as np

# ---- problem sizes (hardcoded per contract) ----
N = 12800
E = 51200
H = 256
F1 = 1568
F2 = 224
NG = 64
NCORES = 8
EPS = 1e-5

# ---- sharding capacities ----
NCAP = 2048          # padded local nodes per core
NNC = NCAP // 128    # 16 node chunks
ECAP = 7168          # padded local edges per core
NKC = ECAP // 128    # 56 edge chunks
W = 7                # scatter windows (edge chunks) per node chunk
NGC = 16             # local graph slots per core
F1PAD = 1664         # F1 padded to 13*128 (weights only)
KF1 = 13
KF1_LAST = 32        # rows in last f1 K-chunk (1568 - 12*128)
KF2 = 2
KF2_LAST = 96        # rows in last f2 K-chunk (224 - 128)

DYNAMIC_WINDOWS = False  # dynamic APs crash HW via this path; use static slack
WSTAT = 13               # static window width fallback
S_BF16 = False           # one-hot scatter matrices in bf16

_PROG_CACHE = {}


# ======================================================================
# Device program
# ======================================================================

def _build_program(sim_compat=False):
    import concourse.bass as bass
    import concourse.mybir as mybir
    import concourse.tile as tile
    from concourse import bacc
    from concourse.bass import OrderedSet, ds
    from concourse.masks import make_identity

    f32 = mybir.dt.float32
    f32r = mybir.dt.float32r
    bf16 = mybir.dt.bfloat16
    s_dt = bf16 if S_BF16 else f32
    AF = mybir.ActivationFunctionType
    WIN = W if DYNAMIC_WINDOWS else WSTAT

    def r(ap):  # matmul operands are f32r-typed end to end
        return ap

    nc = bacc.Bacc("TRN2", target_bir_lowering=False, debug=False,
                   num_devices=NCORES)

    def din(name, shape, dt=f32r):
        return nc.dram_tensor(name, shape, dt, kind="ExternalInput")

    # -- data shards
    f1t_d = din("f1t", [NKC, F1, 128])
    f2t_d = din("f2t", [NKC, F2, 128])
    xsrct_d = din("xsrct", [NKC, H, 128])
    xloct_d = din("xloct", [H, NCAP])
    s_d = din("s_oh", [NNC, WIN, 128, 128], s_dt if S_BF16 else f32r)
    widx_d = din("widx", [1, NNC], mybir.dt.int32)
    g_d = din("g_oh", [NNC, 128, NGC])
    gt_d = din("gt_oh", [NGC, NCAP])
    # -- weights (replicated)
    w1p_d = din("w1p", [H, F1PAD])
    w2t1_d = din("w2t1", [H, H])
    w1p2_d = din("w1p2", [H, H])          # f2_w1 zero-padded 224->256
    w2t2_d = din("w2t2", [H, H])
    linwt_d = din("linwt", [H, H])
    c1llt_d = din("c1llt", [H, H])
    c1lrt_d = din("c1lrt", [H, H])
    c2llt_d = din("c2llt", [H, H])
    c2lrt_d = din("c2lrt", [H, H])
    lin1t_d = din("lin1t", [H, H])
    lin2t_d = din("lin2t", [H, H])
    lincatt_d = din("lincatt", [2 * H, H])
    linst_d = din("linst", [3 * H, H])
    finalt_d = din("finalt", [H, H])
    linb_row_d = din("linb_row", [1, H])
    linb_pp_d = din("linb_pp", [128, 2], f32)
    c1llb_pp_d = din("c1llb_pp", [128, 2], f32)
    c2llb_pp_d = din("c2llb_pp", [128, 2], f32)
    lin1b_pp_d = din("lin1b_pp", [128, 2], f32)
    lin2b_pp_d = din("lin2b_pp", [128, 2], f32)
    lincatb_pp_d = din("lincatb_pp", [128, 2], f32)
    linsb_pp_d = din("linsb_pp", [128, 6], f32)
    finalb_pp_d = din("finalb_pp", [128, 2], f32)
    gamma_pp_d = din("gamma_pp", [128, 2], f32)
    beta_pp_d = din("beta_pp", [128, 2], f32)
    alpha_row_d = din("alpha_row", [1, H], f32)
    ones_d = din("ones", [1, 128])

    outt_d = nc.dram_tensor("outt", [H, NCAP], f32, kind="ExternalOutput")

    from contextlib import ExitStack

    MRING = 16  # message ring chunks (>= WSTAT)

    # static per-node-chunk window starts (must match _shard)
    kstart = [max(0, min(4 * c - (WIN - 5) // 2, NKC - WIN)) for c in range(NNC)]
    # node chunks to scatter once edge chunk k is produced
    trigger = {k: [] for k in range(NKC)}
    for c in range(NNC):
        trigger[kstart[c] + WIN - 1].append(c)

    with tile.TileContext(nc) as tc, ExitStack() as stack:
        const = stack.enter_context(tc.tile_pool(name="const", bufs=1))
        big = stack.enter_context(tc.tile_pool(name="big", bufs=1))
        stream = stack.enter_context(tc.tile_pool(name="stream", bufs=2))
        s3 = stack.enter_context(tc.tile_pool(name="s3", bufs=3))
        spool = stack.enter_context(tc.tile_pool(name="spool", bufs=4))
        psum = stack.enter_context(tc.tile_pool(name="psum", bufs=1, space="PSUM"))
        psum2 = stack.enter_context(tc.tile_pool(name="psum2", bufs=2, space="PSUM"))

        def load_w2(d, pool=None):  # [256, X] -> sbuf [128, 2, X]
            t = (pool or const).tile([128, 2, d.shape[1]], f32r, tag=f"w_{d.name}")
            nc.sync.dma_start(out=t[:], in_=d[:].rearrange("(a p) n -> p a n", p=128))
            return t

        def load_pp(d):
            t = const.tile([128, d.shape[1]], f32, tag=f"pp_{d.name}")
            nc.sync.dma_start(out=t[:], in_=d[:])
            return t

        def silu_act(out_ap, in_ap, bias_ap=None):
            if not sim_compat:
                if bias_ap is None:
                    nc.scalar.activation(out=out_ap, in_=in_ap, func=AF.Silu)
                else:
                    nc.scalar.activation(out=out_ap, in_=in_ap, func=AF.Silu,
                                         bias=bias_ap)
                return
            shp = [out_ap.shape[0], out_ap.shape[-1]]
            t = stream.tile(shp, f32, tag="silutmp")
            if bias_ap is None:
                nc.vector.tensor_copy(out=t[:], in_=in_ap)
            else:
                nc.scalar.activation(out=t[:], in_=in_ap, func=AF.Identity,
                                     bias=bias_ap)
            s = stream.tile(shp, f32, tag="silutmp2")
            nc.scalar.activation(out=s[:], in_=t[:], func=AF.Sigmoid)
            nc.vector.tensor_tensor(out=out_ap, in0=t[:], in1=s[:],
                                    op=mybir.AluOpType.mult)

        linwt = load_w2(linwt_d)
        c1llt = load_w2(c1llt_d)
        c1lrt = load_w2(c1lrt_d)
        c2llt = load_w2(c2llt_d)
        c2lrt = load_w2(c2lrt_d)
        lin1t = load_w2(lin1t_d)
        lin2t = load_w2(lin2t_d)
        finalt = load_w2(finalt_d)
        lincatt = const.tile([128, 4, H], f32r)
        nc.sync.dma_start(out=lincatt[:], in_=lincatt_d[:].rearrange("(a p) n -> p a n", p=128))
        linst = const.tile([128, 6, H], f32r)
        nc.sync.dma_start(out=linst[:], in_=linst_d[:].rearrange("(a p) n -> p a n", p=128))

        linb_pp = load_pp(linb_pp_d)
        c1llb_pp = load_pp(c1llb_pp_d)
        c2llb_pp = load_pp(c2llb_pp_d)
        lin1b_pp = load_pp(lin1b_pp_d)
        lin2b_pp = load_pp(lin2b_pp_d)
        lincatb_pp = load_pp(lincatb_pp_d)
        linsb_pp = load_pp(linsb_pp_d)
        finalb_pp = load_pp(finalb_pp_d)
        gamma_pp = load_pp(gamma_pp_d)
        beta_pp = load_pp(beta_pp_d)

        linb_bc = const.tile([128, H], f32)
        nc.sync.dma_start(out=linb_bc[:], in_=linb_row_d[:].bitcast(f32).to_broadcast((128, H)))
        alpha16 = const.tile([NGC, H], f32)
        nc.sync.dma_start(out=alpha16[:], in_=alpha_row_d[:].to_broadcast((NGC, H)))

        ident = const.tile([128, 128], f32)
        make_identity(nc, ident[:])

        g_oh = const.tile([128, NNC, NGC], f32r)
        nc.sync.dma_start(out=g_oh[:], in_=g_d[:].rearrange("c p g -> p c g"))
        gt_oh = const.tile([NGC, NCAP], f32r)
        nc.sync.dma_start(out=gt_oh[:], in_=gt_d[:])

        # ---- fold combined edge-MLP weights: WcT = W1T @ W2T (streamed) ----
        wc1t = const.tile([128, KF1, H], f32r)
        wc2t = const.tile([128, KF2, H], f32r)
        w2t1_sb = load_w2(w2t1_d, pool=stream)
        for fk in range(KF1):
            wtile = stream.tile([128, 2, 128], f32r, tag="wfold")
            nc.sync.dma_start(out=wtile[:],
                              in_=w1p_d[:, fk * 128:(fk + 1) * 128].rearrange("(a p) f -> p a f", p=128))
            ps = psum.tile([128, H], f32, tag="psf")
            for hc in range(2):
                nc.tensor.matmul(ps[:], lhsT=wtile[:, hc, :], rhs=w2t1_sb[:, hc, :],
                                 start=(hc == 0), stop=(hc == 1))
            nc.vector.tensor_copy(out=wc1t[:, fk, :], in_=ps[:])
        w2t2_sb = load_w2(w2t2_d, pool=stream)
        for fk in range(KF2):
            wtile = stream.tile([128, 2, 128], f32r, tag="wfold")
            nc.sync.dma_start(out=wtile[:],
                              in_=w1p2_d[:, fk * 128:(fk + 1) * 128].rearrange("(a p) f -> p a f", p=128))
            ps = psum.tile([128, H], f32, tag="psf")
            for hc in range(2):
                nc.tensor.matmul(ps[:], lhsT=wtile[:, hc, :], rhs=w2t2_sb[:, hc, :],
                                 start=(hc == 0), stop=(hc == 1))
            nc.vector.tensor_copy(out=wc2t[:, fk, :], in_=ps[:])

        # ---- x_local projection (H-major, fused bias+swish) ----
        xlocT = big.tile([128, 2, NCAP], f32r, tag="xlocT")
        for n4 in range(NCAP // 512):
            xlr = s3.tile([128, 2, 512], f32r, tag="n2x512")
            nc.sync.dma_start(out=xlr[:],
                              in_=xloct_d[:, n4 * 512:(n4 + 1) * 512].rearrange("(a p) n -> p a n", p=128))
            for ho in range(2):
                ps = psum2.tile([128, 512], f32, tag="nodeps")
                for hc in range(2):
                    nc.tensor.matmul(ps[:], lhsT=linwt[:, hc, ho * 128:(ho + 1) * 128],
                                     rhs=xlr[:, hc, :],
                                     start=(hc == 0), stop=(hc == 1))
                silu_act(xlocT[:, ho, n4 * 512:(n4 + 1) * 512], ps[:],
                         linb_pp[:, ho:ho + 1])

        # ---- branches: messages (ring) + scatter + conv + streamed lincat ----
        msgs = big.tile([128, MRING, H], f32r, tag="msgs")
        aggT = big.tile([128, 2, NCAP], f32r, tag="aggT")
        hcat = big.tile([128, 2, NCAP], f32r, tag="hcat")
        hT = big.tile([128, 2, NCAP], f32r, tag="hT")

        for br in range(2):
            if br == 0:
                ft_d, KF, KLAST, wct = f1t_d, KF1, KF1_LAST, wc1t
                cllt, clrt, clb = c1llt, c1lrt, c1llb_pp
                lint, linb_b = lin1t, lin1b_pp
            else:
                ft_d, KF, KLAST, wct = f2t_d, KF2, KF2_LAST, wc2t
                cllt, clrt, clb = c2llt, c2lrt, c2llb_pp
                lint, linb_b = lin2t, lin2b_pp
            KFULL = KF - 1

            def produce_chunk(k):
                ftile = stream.tile([128, KF, 128], f32r, tag=f"ftile{br}")
                nc.sync.dma_start(
                    out=ftile[:, :KFULL, :],
                    in_=ft_d[k, :KFULL * 128, :].rearrange("(o p) f -> p o f", p=128))
                nc.sync.dma_start(out=ftile[:KLAST, KFULL, :],
                                  in_=ft_d[k, KFULL * 128:, :])
                ps_f = psum.tile([128, H], f32, tag="psf")
                for kc in range(KF):
                    kk = 128 if kc < KFULL else KLAST
                    nc.tensor.matmul(ps_f[:], lhsT=ftile[:kk, kc, :],
                                     rhs=wct[:kk, kc, :],
                                     start=(kc == 0), stop=(kc == KF - 1))
                xstile = stream.tile([128, 2, 128], f32r, tag="xstile")
                nc.sync.dma_start(out=xstile[:],
                                  in_=xsrct_d[k].rearrange("(a p) e -> p a e", p=128))
                ps_x = psum.tile([128, H], f32, tag="psx")
                nc.tensor.matmul(ps_x[:], lhsT=xstile[:, 0, :], rhs=linwt[:, 0, :],
                                 start=True, stop=False)
                nc.tensor.matmul(ps_x[:], lhsT=xstile[:, 1, :], rhs=linwt[:, 1, :],
                                 start=False, stop=True)
                xs = stream.tile([128, H], f32r, tag="xs")
                nc.vector.tensor_add(out=xs[:].bitcast(f32), in0=ps_x[:], in1=linb_bc[:])
                silu_act(xs[:], xs[:].bitcast(f32))
                nc.vector.tensor_mul(out=msgs[:, k % MRING, :], in0=ps_f[:], in1=xs[:])

            def scatter_chunk(c):
                ps_a = psum.tile([128, H], f32, tag="psagg")
                for w in range(WIN):
                    kk = kstart[c] + w
                    s_tile = spool.tile([128, 128], s_dt if S_BF16 else f32r, tag="s_oh")
                    nc.sync.dma_start(out=s_tile[:], in_=s_d[c, w])
                    nc.tensor.matmul(ps_a[:], lhsT=s_tile[:],
                                     rhs=msgs[:, kk % MRING, :],
                                     start=(w == 0), stop=(w == WIN - 1))
                agg_nm = stream.tile([128, H], f32, tag="aggnm")
                nc.vector.tensor_copy(out=agg_nm[:], in_=ps_a[:])
                for hc in range(2):
                    ps_t = psum.tile([128, 128], f32, tag="pst")
                    nc.tensor.transpose(ps_t[:], agg_nm[:, hc * 128:(hc + 1) * 128], ident[:])
                    nc.vector.tensor_copy(out=aggT[:, hc, c * 128:(c + 1) * 128], in_=ps_t[:])

            for k in range(NKC):
                produce_chunk(k)
                for c in trigger[k]:
                    scatter_chunk(c)

            # conv + lin + streamed lincat accumulation
            for n4 in range(NCAP // 512):
                nsl = slice(n4 * 512, (n4 + 1) * 512)
                inner = s3.tile([128, 2, 512], f32r, tag="n2x512")
                for ho in range(2):
                    hsl = slice(ho * 128, (ho + 1) * 128)
                    ps = psum2.tile([128, 512], f32, tag="nodeps")
                    nc.tensor.matmul(ps[:], lhsT=cllt[:, 0, hsl], rhs=aggT[:, 0, nsl],
                                     start=True, stop=False)
                    nc.tensor.matmul(ps[:], lhsT=cllt[:, 1, hsl], rhs=aggT[:, 1, nsl],
                                     start=False, stop=False)
                    nc.tensor.matmul(ps[:], lhsT=clrt[:, 0, hsl], rhs=xlocT[:, 0, nsl],
                                     start=False, stop=False)
                    nc.tensor.matmul(ps[:], lhsT=clrt[:, 1, hsl], rhs=xlocT[:, 1, nsl],
                                     start=False, stop=True)
                    nc.scalar.activation(out=inner[:, ho, :], in_=ps[:], func=AF.Identity,
                                         bias=clb[:, ho:ho + 1])
                hb = s3.tile([128, 2, 512], f32r, tag="n2x512")
                for ho in range(2):
                    hsl = slice(ho * 128, (ho + 1) * 128)
                    ps2 = psum.tile([128, 512], f32, tag="nodeps2")
                    for hc in range(2):
                        nc.tensor.matmul(ps2[:], lhsT=lint[:, hc, hsl],
                                         rhs=inner[:, hc, :],
                                         start=(hc == 0), stop=(hc == 1))
                    silu_act(hb[:, ho, :], ps2[:], linb_b[:, ho:ho + 1])
                # lincat partial: hcat (br0: init, br1: accumulate + finish hT)
                for ho in range(2):
                    hsl = slice(ho * 128, (ho + 1) * 128)
                    ps3 = psum.tile([128, 512], f32, tag="cat")
                    for hc in range(2):
                        nc.tensor.matmul(ps3[:], lhsT=lincatt[:, br * 2 + hc, hsl],
                                         rhs=hb[:, hc, :],
                                         start=(hc == 0), stop=(hc == 1))
                    if br == 0:
                        nc.vector.tensor_copy(out=hcat[:, ho, nsl], in_=ps3[:])
                    else:
                        tmp = stream.tile([128, 512], f32, tag="tmp512")
                        nc.vector.tensor_add(out=tmp[:], in0=ps3[:], in1=hcat[:, ho, nsl])
                        nc.scalar.activation(out=tmp[:], in_=tmp[:], func=AF.Identity,
                                             bias=lincatb_pp[:, ho:ho + 1])
                        nc.vector.tensor_add(out=hT[:, ho, nsl], in0=tmp[:],
                                             in1=xlocT[:, ho, nsl])

        # ---- residual lins (in place on hT; both ho psums read before writes) ----
        for l in range(3):
            for n4 in range(NCAP // 512):
                nsl = slice(n4 * 512, (n4 + 1) * 512)
                pss = []
                for ho in range(2):
                    hsl = slice(ho * 128, (ho + 1) * 128)
                    ps = psum2.tile([128, 512], f32, tag="nodeps")
                    for hc in range(2):
                        nc.tensor.matmul(ps[:], lhsT=linst[:, l * 2 + hc, hsl],
                                         rhs=hT[:, hc, nsl],
                                         start=(hc == 0), stop=(hc == 1))
                    pss.append(ps)
                for ho in range(2):
                    sw = stream.tile([128, 512], f32, tag="tmp512")
                    silu_act(sw[:], pss[ho][:], linsb_pp[:, l * 2 + ho:l * 2 + ho + 1])
                    nc.vector.tensor_add(out=hT[:, ho, nsl], in0=sw[:], in1=hT[:, ho, nsl])

        # ---- GraphNorm ----
        h_nm = big.tile([128, NNC, H], f32r, tag="xlocT")
        for c in range(NNC):
            for hc in range(2):
                ps_t = psum.tile([128, 128], f32, tag="pst")
                nc.tensor.transpose(ps_t[:], hT[:, hc, c * 128:(c + 1) * 128].bitcast(f32),
                                    ident[:])
                nc.vector.tensor_copy(out=h_nm[:, c, hc * 128:(hc + 1) * 128], in_=ps_t[:])
        sq_nm = big.tile([128, NNC, H], f32r, tag="msgs")
        nc.vector.tensor_mul(out=sq_nm[:], in0=h_nm[:], in1=h_nm[:])

        ps_sh = psum.tile([NGC, H], f32, tag="psf")
        ps_sq = psum.tile([NGC, H], f32, tag="psx")
        for c in range(NNC):
            nc.tensor.matmul(ps_sh[:], lhsT=g_oh[:, c, :], rhs=h_nm[:, c, :],
                             start=(c == 0), stop=(c == NNC - 1))
            nc.tensor.matmul(ps_sq[:], lhsT=g_oh[:, c, :], rhs=sq_nm[:, c, :],
                             start=(c == 0), stop=(c == NNC - 1))
        cnt = const.tile([NGC, 1], f32)
        nc.vector.tensor_reduce(cnt[:], gt_oh[:].bitcast(f32), axis=mybir.AxisListType.X,
                                op=mybir.AluOpType.add)
        inv_cnt = const.tile([NGC, 1], f32)
        nc.vector.tensor_scalar_max(inv_cnt[:], cnt[:], 1.0)
        nc.vector.reciprocal(out=inv_cnt[:], in_=inv_cnt[:])
        mean = const.tile([NGC, H], f32)
        nc.vector.tensor_tensor(out=mean[:], in0=ps_sh[:],
                                in1=inv_cnt[:].to_broadcast((NGC, H)),
                                op=mybir.AluOpType.mult)
        meansq = const.tile([NGC, H], f32)
        nc.vector.tensor_tensor(out=meansq[:], in0=ps_sq[:],
                                in1=inv_cnt[:].to_broadcast((NGC, H)),
                                op=mybir.AluOpType.mult)
        am = const.tile([NGC, H], f32r)
        nc.vector.tensor_mul(out=am[:], in0=alpha16[:], in1=mean[:])
        t2m = const.tile([NGC, H], f32)
        nc.vector.tensor_scalar_mul(t2m[:], mean[:], 2.0)
        nc.vector.tensor_sub(out=t2m[:], in0=t2m[:], in1=am[:].bitcast(f32))
        nc.vector.tensor_mul(out=t2m[:], in0=am[:].bitcast(f32), in1=t2m[:])
        var = const.tile([NGC, H], f32)
        nc.vector.tensor_sub(out=var[:], in0=meansq[:], in1=t2m[:])
        nc.vector.tensor_scalar_add(var[:], var[:], float(EPS))
        std = const.tile([NGC, H], f32)
        nc.scalar.activation(out=std[:], in_=var[:], func=AF.Sqrt)
        rstd32 = const.tile([NGC, H], f32)
        nc.vector.reciprocal(out=rstd32[:], in_=std[:])
        rstd = const.tile([NGC, H], f32r)
        nc.vector.tensor_copy(out=rstd[:], in_=rstd32[:])

        hnT = big.tile([128, 2, NCAP], f32r, tag="aggT")
        for n4 in range(NCAP // 512):
            nsl = slice(n4 * 512, (n4 + 1) * 512)
            for ho in range(2):
                hsl = slice(ho * 128, (ho + 1) * 128)
                ps_am = psum.tile([128, 512], f32, tag="nodeps2")
                nc.tensor.matmul(ps_am[:], lhsT=am[:, hsl], rhs=gt_oh[:, nsl],
                                 start=True, stop=True)
                ps_rs = psum.tile([128, 512], f32, tag="cat")
                nc.tensor.matmul(ps_rs[:], lhsT=rstd[:, hsl], rhs=gt_oh[:, nsl],
                                 start=True, stop=True)
                t = stream.tile([128, 512], f32, tag="tmp512")
                nc.vector.tensor_sub(out=t[:], in0=hT[:, ho, nsl], in1=ps_am[:])
                nc.vector.tensor_mul(out=t[:], in0=t[:], in1=ps_rs[:])
                nc.scalar.activation(out=hnT[:, ho, nsl], in_=t[:], func=AF.Identity,
                                     scale=gamma_pp[:, ho:ho + 1],
                                     bias=beta_pp[:, ho:ho + 1])

        # ---- final linear ----
        outt_r = outt_d[:].rearrange("(a p) n -> p a n", p=128)
        for n4 in range(NCAP // 512):
            nsl = slice(n4 * 512, (n4 + 1) * 512)
            for ho in range(2):
                hsl = slice(ho * 128, (ho + 1) * 128)
                ps = psum2.tile([128, 512], f32, tag="nodeps")
                for hc in range(2):
                    nc.tensor.matmul(ps[:], lhsT=finalt[:, hc, hsl],
                                     rhs=hnT[:, hc, nsl],
                                     start=(hc == 0), stop=(hc == 1))
                ot = stream.tile([128, 512], f32, tag="tmp512")
                nc.scalar.activation(out=ot[:], in_=ps[:], func=AF.Identity,
                                     bias=finalb_pp[:, ho:ho + 1])
                nc.sync.dma_start(out=outt_r[:, ho, nsl], in_=ot[:])

    nc.compile()
    return nc


def _get_program(sim_compat=False):
    key = ("sim" if sim_compat else "hw")
    if key not in _PROG_CACHE:
        _PROG_CACHE[key] = _build_program(sim_compat)
    return _PROG_CACHE[key]


# ======================================================================
# Host-side sharding
# ======================================================================

def _pp(b):  # [256] -> per-partition [128, 2] (ho-chunk columns)
    return np.ascontiguousarray(b.reshape(2, 128).T, dtype=np.float32)


def _shared_weights(inp):
    f32 = np.float32
    w = {}
    w1p = np.zeros((H, F1PAD), f32)
    w1p[:, :F1] = inp["f1_w1"]
    w["w1p"] = w1p
    w["w2t1"] = np.ascontiguousarray(inp["f1_w2"].T, f32)
    w1p2 = np.zeros((H, H), f32)
    w1p2[:, :F2] = inp["f2_w1"]
    w["w1p2"] = w1p2
    w["w2t2"] = np.ascontiguousarray(inp["f2_w2"].T, f32)
    for name, key in [("linwt", "lin_w"), ("c1llt", "c1_ll_w"), ("c1lrt", "c1_lr_w"),
                      ("c2llt", "c2_ll_w"), ("c2lrt", "c2_lr_w"),
                      ("lin1t", "lin1_w"), ("lin2t", "lin2_w"), ("finalt", "final_w")]:
        w[name] = np.ascontiguousarray(np.asarray(inp[key], f32).T)
    w["lincatt"] = np.ascontiguousarray(np.asarray(inp["lincat_w"], f32).T)  # [512,256]
    w["linst"] = np.ascontiguousarray(
        np.concatenate([np.asarray(inp["lins_w"][l], f32).T for l in range(3)], axis=0))
    w["linb_row"] = np.asarray(inp["lin_b"], f32).reshape(1, H).copy()
    w["linb_pp"] = _pp(np.asarray(inp["lin_b"], f32))
    w["c1llb_pp"] = _pp(np.asarray(inp["c1_ll_b"], f32))
    w["c2llb_pp"] = _pp(np.asarray(inp["c2_ll_b"], f32))
    w["lin1b_pp"] = _pp(np.asarray(inp["lin1_b"], f32))
    w["lin2b_pp"] = _pp(np.asarray(inp["lin2_b"], f32))
    w["lincatb_pp"] = _pp(np.asarray(inp["lincat_b"], f32))
    w["linsb_pp"] = np.concatenate(
        [_pp(np.asarray(inp["lins_b"][l], f32)) for l in range(3)], axis=1)  # [128, 6]
    w["finalb_pp"] = _pp(np.asarray(inp["final_b"], f32))
    w["gamma_pp"] = _pp(np.asarray(inp["norm_gamma"], f32))
    w["beta_pp"] = _pp(np.asarray(inp["norm_beta"], f32))
    w["alpha_row"] = np.asarray(inp["norm_alpha"], f32).reshape(1, H).copy()
    return w


def _shard(inp):
    f32 = np.float32
    x = np.asarray(inp["x"], f32)
    f1 = np.asarray(inp["feature1"], f32)
    f2 = np.asarray(inp["feature2"], f32)
    ei = np.asarray(inp["edge_index"]).astype(np.int64)
    batch = np.asarray(inp["batch"]).astype(np.int64)
    src, tgt = ei[0], ei[1]

    gn_counts = np.bincount(batch, minlength=NG)          # nodes per graph
    ge_counts = np.bincount(batch[tgt], minlength=NG)     # edges per graph (by target)
    gn_start = np.concatenate([[0], np.cumsum(gn_counts)])

    # contiguous graph partition balancing edges
    cume = np.cumsum(ge_counts)
    bounds = [0]
    for c in range(1, NCORES):
        target = cume[-1] * c / NCORES
        g = int(np.searchsorted(cume, target))
        bounds.append(max(bounds[-1] + 1, min(g + 1, NG - (NCORES - c))))
    bounds.append(NG)

    sdt = np.dtype("bfloat16") if S_BF16 else f32
    if S_BF16:
        import ml_dtypes
        sdt = ml_dtypes.bfloat16

    w = _shared_weights(inp)
    in_maps = []
    meta = []
    WIN = W if DYNAMIC_WINDOWS else WSTAT
    for c in range(NCORES):
        glo, ghi = bounds[c], bounds[c + 1]
        ns, ne = int(gn_start[glo]), int(gn_start[ghi])
        ncnt = ne - ns
        assert ncnt <= NCAP, f"core {c}: {ncnt} nodes > NCAP"
        assert ghi - glo <= NGC, f"core {c}: {ghi - glo} graphs > NGC"

        emask = (tgt >= ns) & (tgt < ne)
        eidx = np.nonzero(emask)[0]
        loc_t = tgt[eidx] - ns
        order = np.argsort(loc_t, kind="stable")
        eidx = eidx[order]
        loc_t = loc_t[order]
        ecnt = len(eidx)
        assert ecnt <= ECAP, f"core {c}: {ecnt} edges > ECAP"

        f1_sh = np.zeros((ECAP, F1), f32)
        f1_sh[:ecnt] = f1[eidx]
        f1t = np.ascontiguousarray(f1_sh.reshape(NKC, 128, F1).transpose(0, 2, 1))
        f2_sh = np.zeros((ECAP, F2), f32)
        f2_sh[:ecnt] = f2[eidx]
        f2t = np.ascontiguousarray(f2_sh.reshape(NKC, 128, F2).transpose(0, 2, 1))
        xs_sh = np.zeros((ECAP, H), f32)
        xs_sh[:ecnt] = x[src[eidx]]
        xsrct = np.ascontiguousarray(xs_sh.reshape(NKC, 128, H).transpose(0, 2, 1))
        xloc = np.zeros((NCAP, H), f32)
        xloc[:ncnt] = x[ns:ne]
        xloct = np.ascontiguousarray(xloc.T)

        cum = np.searchsorted(loc_t, np.arange(NNC) * 128)
        if DYNAMIC_WINDOWS:
            kstart = np.clip(cum // 128, 0, NKC - W)
        else:
            kstart = np.clip(4 * np.arange(NNC) - (WSTAT - 5) // 2, 0, NKC - WSTAT)
        slots = np.arange(ecnt)
        kk = slots // 128
        cc = loc_t // 128
        ww = kk - kstart[cc]
        assert (ww >= 0).all() and (ww < WIN).all(), f"core {c}: window overflow"
        s_oh = np.zeros((NNC, WIN, 128, 128), sdt)
        s_oh[cc, ww, slots % 128, loc_t - cc * 128] = 1
        widx = (kstart * H).astype(np.int32).reshape(1, NNC)

        g_loc = batch[ns:ne] - glo
        nl = np.arange(ncnt)
        g_oh = np.zeros((NNC, 128, NGC), f32)
        g_oh[nl // 128, nl % 128, g_loc] = 1
        gt_oh = np.zeros((NGC, NCAP), f32)
        gt_oh[g_loc, nl] = 1

        m = {"f1t": f1t, "f2t": f2t, "xsrct": xsrct, "xloct": xloct,
             "s_oh": s_oh, "widx": widx, "g_oh": g_oh, "gt_oh": gt_oh,
             "ones": np.ones((1, 128), f32)}
        m.update(w)
        in_maps.append(m)
        meta.append((ns, ne))
    return in_maps, meta


def kernel(**inputs):
    from concourse.bass_utils import run_bass_kernel_spmd

    nc = _get_program()
    in_maps, meta = _shard(inputs)
    res = run_bass_kernel_spmd(nc, in_maps, list(range(NCORES)))
    out = np.empty((N, H), np.float32)
    for c, (ns, ne) in enumerate(meta):
        out[ns:ne] = res.results[c]["outt"][:, :ne - ns].T
    return out


# revision 23
# speedup vs baseline: 181.5453x; 181.5453x over previous
"""ComENet-style GNN block on 8 Trainium2 NeuronCores (Bass/Tile SPMD).

Sharding: nodes/edges by graph (contiguous graphs per core, cut to balance
edge counts). Edges assigned to the core owning their TARGET node and sorted
by local target, so scatter stays on-device via one-hot matmuls. Source-node
features are host-gathered per edge (pure input rearrangement); all model
arithmetic runs on device. Weights replicated across cores.

Per-core device program (all matmuls fp32r = full-precision fp32 at bf16 rate):
  1. fold W2@W1 for both edge-feature MLPs (device matmul)
  2. x_local = swish(x @ lin_w.T + b)  (H-major)
  3. per branch: msgs[e] = (feat[e] @ Wc.T) * swish(x_src[e] @ lin_w.T + b)
     (edge-major), scatter = one-hot S matmuls over register-indexed dynamic
     message windows -> agg (node-major) -> PE-transpose -> H-major
  4. conv + lin1/lin2, lincat + residual, 3 residual lins (H-major)
  5. GraphNorm via per-graph moment matmuls (exact: var = E[h^2]-2a m^2+a^2 m^2)
  6. final linear -> out^T, host reassembles [N, H]
"""

import os

os.environ.setdefault("MYCRO_LOCAL_CACHE", "1")

import numpy as np

# ---- problem sizes (hardcoded per contract) ----
N = 12800
E = 51200
H = 256
F1 = 1568
F2 = 224
NG = 64
NCORES = 8
EPS = 1e-5

# ---- sharding capacities ----
NCAP = 2048          # padded local nodes per core
NNC = NCAP // 128    # 16 node chunks
ECAP = 7168          # padded local edges per core
NKC = ECAP // 128    # 56 edge chunks
W = 7                # (unused in static mode)
NGC = 16             # local graph slots per core
KP = 112             # matmul K-chunk rows (F1 = 14*112, F2 = 2*112)
KF1 = 14
KF2 = 2

DYNAMIC_WINDOWS = False  # dynamic APs crash HW via this path; use static slack
WSTAT = 9                # static window width (kstart = clip(4c-2, 0, NKC-9))
S_BF16 = False           # one-hot scatter matrices in bf16

_PROG_CACHE = {}


# ======================================================================
# Device program
# ======================================================================

def _build_program(sim_compat=False):
    import concourse.bass as bass
    import concourse.mybir as mybir
    import concourse.tile as tile
    from concourse import bacc
    from concourse.bass import OrderedSet, ds
    from concourse.masks import make_identity

    f32 = mybir.dt.float32
    f32r = mybir.dt.float32r
    bf16 = mybir.dt.bfloat16
    s_dt = bf16 if S_BF16 else f32
    AF = mybir.ActivationFunctionType
    WIN = W if DYNAMIC_WINDOWS else WSTAT

    def r(ap):  # matmul operands are f32r-typed end to end
        return ap

    nc = bacc.Bacc("TRN2", target_bir_lowering=False, debug=False,
                   num_devices=NCORES)

    def din(name, shape, dt=f32r):
        return nc.dram_tensor(name, shape, dt, kind="ExternalInput")

    # -- data shards
    f1t_d = din("f1t", [NKC, KP, KF1 * 128])
    f2t_d = din("f2t", [NKC, KP, KF2 * 128])
    xsrct_d = din("xsrct", [NKC, 128, 2 * 128])
    xloct_d = din("xloct", [H, NCAP])
    s_d = din("s_oh", [NNC, 128, WIN, 128], s_dt if S_BF16 else f32r)
    widx_d = din("widx", [1, NNC], mybir.dt.int32)
    g_d = din("g_oh", [NNC, 128, NGC])
    gt_d = din("gt_oh", [NGC, NCAP])
    # -- weights (replicated)
    w1_d = din("w1", [H, F1])
    w2t1_d = din("w2t1", [H, H])
    w12_d = din("w12", [H, F2])
    w2t2_d = din("w2t2", [H, H])
    linwt_d = din("linwt", [H, H])
    c1llt_d = din("c1llt", [H, H])
    c1lrt_d = din("c1lrt", [H, H])
    c2llt_d = din("c2llt", [H, H])
    c2lrt_d = din("c2lrt", [H, H])
    lin1t_d = din("lin1t", [H, H])
    lin2t_d = din("lin2t", [H, H])
    lincatt_d = din("lincatt", [2 * H, H])
    linst_d = din("linst", [3 * H, H])
    finalt_d = din("finalt", [H, H])
    linb_row_d = din("linb_row", [1, H])
    linb_pp_d = din("linb_pp", [128, 2], f32)
    c1llb_pp_d = din("c1llb_pp", [128, 2], f32)
    c2llb_pp_d = din("c2llb_pp", [128, 2], f32)
    lin1b_pp_d = din("lin1b_pp", [128, 2], f32)
    lin2b_pp_d = din("lin2b_pp", [128, 2], f32)
    lincatb_pp_d = din("lincatb_pp", [128, 2], f32)
    linsb_pp_d = din("linsb_pp", [128, 6], f32)
    finalb_pp_d = din("finalb_pp", [128, 2], f32)
    gamma_pp_d = din("gamma_pp", [128, 2], f32)
    beta_pp_d = din("beta_pp", [128, 2], f32)
    alpha_row_d = din("alpha_row", [1, H], f32)
    ones_d = din("ones", [1, 128])

    outt_d = nc.dram_tensor("outt", [H, NCAP], f32, kind="ExternalOutput")

    from contextlib import ExitStack

    with tile.TileContext(nc) as tc, ExitStack() as stack:
        const = stack.enter_context(tc.tile_pool(name="const", bufs=1))
        big = stack.enter_context(tc.tile_pool(name="big", bufs=1))
        stream = stack.enter_context(tc.tile_pool(name="stream", bufs=2))
        s3 = stack.enter_context(tc.tile_pool(name="s3", bufs=3))
        spool = stack.enter_context(tc.tile_pool(name="spool", bufs=2))
        psum = stack.enter_context(tc.tile_pool(name="psum", bufs=1, space="PSUM"))
        psumd = stack.enter_context(tc.tile_pool(name="psumd", bufs=2, space="PSUM"))

        def load_w2(d, pool=None):  # [256, X] -> sbuf [128, 2, X]
            t = (pool or const).tile([128, 2, d.shape[1]], f32r, tag=f"w_{d.name}")
            nc.sync.dma_start(out=t[:], in_=d[:].rearrange("(a p) n -> p a n", p=128))
            return t

        def load_pp(d):
            t = const.tile([128, d.shape[1]], f32, tag=f"pp_{d.name}")
            nc.sync.dma_start(out=t[:], in_=d[:])
            return t

        def silu_act(out_ap, in_ap, bias_ap=None):
            if not sim_compat:
                if bias_ap is None:
                    nc.scalar.activation(out=out_ap, in_=in_ap, func=AF.Silu)
                else:
                    nc.scalar.activation(out=out_ap, in_=in_ap, func=AF.Silu,
                                         bias=bias_ap)
                return
            shp = [out_ap.shape[0], out_ap.shape[-1]]
            t = s3.tile([128, 512], f32, tag="n2x512", name="silt")[:shp[0], :shp[1]]
            if bias_ap is None:
                nc.vector.tensor_copy(out=t[:], in_=in_ap)
            else:
                nc.scalar.activation(out=t[:], in_=in_ap, func=AF.Identity,
                                     bias=bias_ap)
            s = s3.tile([128, 512], f32, tag="n2x512", name="sils")[:shp[0], :shp[1]]
            nc.scalar.activation(out=s[:], in_=t[:], func=AF.Sigmoid)
            nc.vector.tensor_tensor(out=out_ap, in0=t[:], in1=s[:],
                                    op=mybir.AluOpType.mult)

        linwt = load_w2(linwt_d)
        c1llt = load_w2(c1llt_d)
        c1lrt = load_w2(c1lrt_d)
        c2llt = load_w2(c2llt_d)
        c2lrt = load_w2(c2lrt_d)
        lin1t = load_w2(lin1t_d)
        lin2t = load_w2(lin2t_d)
        finalt = load_w2(finalt_d)
        lincatt = const.tile([128, 4, H], f32r)
        nc.sync.dma_start(out=lincatt[:], in_=lincatt_d[:].rearrange("(a p) n -> p a n", p=128))
        linst = const.tile([128, 6, H], f32r)
        nc.sync.dma_start(out=linst[:], in_=linst_d[:].rearrange("(a p) n -> p a n", p=128))

        linb_pp = load_pp(linb_pp_d)
        c1llb_pp = load_pp(c1llb_pp_d)
        c2llb_pp = load_pp(c2llb_pp_d)
        lin1b_pp = load_pp(lin1b_pp_d)
        lin2b_pp = load_pp(lin2b_pp_d)
        lincatb_pp = load_pp(lincatb_pp_d)
        linsb_pp = load_pp(linsb_pp_d)
        finalb_pp = load_pp(finalb_pp_d)
        gamma_pp = load_pp(gamma_pp_d)
        beta_pp = load_pp(beta_pp_d)

        linb_bc = const.tile([128, H], f32)
        nc.sync.dma_start(out=linb_bc[:], in_=linb_row_d[:].bitcast(f32).to_broadcast((128, H)))
        alpha16 = const.tile([NGC, H], f32)
        nc.sync.dma_start(out=alpha16[:], in_=alpha_row_d[:].to_broadcast((NGC, H)))

        ident = const.tile([128, 128], f32)
        make_identity(nc, ident[:])

        g_oh = const.tile([128, NNC, NGC], f32r)
        nc.sync.dma_start(out=g_oh[:], in_=g_d[:].rearrange("c p g -> p c g"))
        gt_oh = const.tile([NGC, NCAP], f32r)
        nc.sync.dma_start(out=gt_oh[:], in_=gt_d[:])

        # ---- fold combined edge-MLP weights: WcT = W1T @ W2T (streamed) ----
        wc1t = const.tile([KP, KF1, H], f32r)
        wc2t = const.tile([KP, KF2, H], f32r)
        w2t1_sb = load_w2(w2t1_d, pool=stream)
        w2t2_sb = load_w2(w2t2_d, pool=stream)
        for wct, wsrc, KF in ((wc1t, w1_d, KF1), (wc2t, w12_d, KF2)):
            for fk in range(KF):
                wtile = stream.tile([128, 2, KP], f32r, tag="wfold")
                nc.sync.dma_start(out=wtile[:],
                                  in_=wsrc[:, fk * KP:(fk + 1) * KP].rearrange("(a p) f -> p a f", p=128))
                ps = psum.tile([KP, H], f32, tag="pE")
                rhs = w2t1_sb if wct is wc1t else w2t2_sb
                for hc in range(2):
                    nc.tensor.matmul(ps[:], lhsT=wtile[:, hc, :], rhs=rhs[:, hc, :],
                                     start=(hc == 0), stop=(hc == 1))
                nc.vector.tensor_copy(out=wct[:, fk, :], in_=ps[:])

        # ---- x_local projection (H-major, fused bias+swish) ----
        xlocT = big.tile([128, 2, NCAP], f32r, tag="xlocT")
        for n4 in range(NCAP // 512):
            xlr = s3.tile([128, 2, 512], f32r, tag="n2x512")
            nc.sync.dma_start(out=xlr[:],
                              in_=xloct_d[:, n4 * 512:(n4 + 1) * 512].rearrange("(a p) n -> p a n", p=128))
            for ho in range(2):
                ps = psum.tile([128, 512], f32, tag="pE")
                for hc in range(2):
                    nc.tensor.matmul(ps[:], lhsT=linwt[:, hc, ho * 128:(ho + 1) * 128],
                                     rhs=xlr[:, hc, :],
                                     start=(hc == 0), stop=(hc == 1))
                silu_act(xlocT[:, ho, n4 * 512:(n4 + 1) * 512], ps[:],
                         linb_pp[:, ho:ho + 1])

        # ---- merged branches: produce msgs for both, scatter both, eager conv ----
        MRING = 10
        kstart = [max(0, min(4 * c - (WSTAT - 5) // 2, NKC - WSTAT)) for c in range(NNC)]
        trigger = {k: [] for k in range(NKC)}
        for c in range(NNC):
            trigger[kstart[c] + WSTAT - 1].append(c)

        msgs1 = big.tile([128, MRING, H], f32r, tag="msgs1")
        msgs2 = big.tile([128, MRING, H], f32r, tag="msgs2")
        hcat = big.tile([128, 2, NCAP], f32r, tag="hcat")
        hT = big.tile([128, 2, NCAP], f32r, tag="hT")
        agg_cur = [None, None]   # rolling [128, 2, 512] aggT tiles per branch

        def produce_chunk(k):
            # branch-1 features (one DMA, 14 K-chunks of 112)
            ftile = stream.tile([KP, KF1, 128], f32r, tag="ftile1")
            nc.sync.dma_start(out=ftile[:],
                              in_=f1t_d[k].rearrange("p (o f) -> p o f", o=KF1))
            ps_f1 = psum.tile([128, H], f32, tag="pA")
            for kc in range(KF1):
                nc.tensor.matmul(ps_f1[:], lhsT=ftile[:, kc, :], rhs=wc1t[:, kc, :],
                                 start=(kc == 0), stop=(kc == KF1 - 1))
            if k % 2 == 0:
                f2pair = stream.tile([KP, 2, KF2, 128], f32r, tag="f2pair")
                nc.sync.dma_start(out=f2pair[:],
                                  in_=f2t_d[k:k + 2].rearrange("b p (o f) -> p b o f", o=KF2))
                xspair = stream.tile([128, 2, 2, 128], f32r, tag="xspair")
                nc.sync.dma_start(out=xspair[:],
                                  in_=xsrct_d[k:k + 2].rearrange("b p (a e) -> p b a e", a=2))
                produce_chunk.f2pair = f2pair
                produce_chunk.xspair = xspair
            f2pair, xspair = produce_chunk.f2pair, produce_chunk.xspair
            b = k % 2
            ps_f2 = psumd.tile([128, H], f32, tag="pB")
            for kc in range(KF2):
                nc.tensor.matmul(ps_f2[:], lhsT=f2pair[:, b, kc, :], rhs=wc2t[:, kc, :],
                                 start=(kc == 0), stop=(kc == KF2 - 1))
            ps_x = psumd.tile([128, H], f32, tag="pB")
            nc.tensor.matmul(ps_x[:], lhsT=xspair[:, b, 0, :], rhs=linwt[:, 0, :],
                             start=True, stop=False)
            nc.tensor.matmul(ps_x[:], lhsT=xspair[:, b, 1, :], rhs=linwt[:, 1, :],
                             start=False, stop=True)
            xs = stream.tile([128, H], f32r, tag="xs")
            nc.vector.tensor_add(out=xs[:].bitcast(f32), in0=ps_x[:], in1=linb_bc[:])
            silu_act(xs[:], xs[:].bitcast(f32))
            nc.vector.tensor_mul(out=msgs1[:, k % MRING, :], in0=ps_f1[:], in1=xs[:])
            nc.vector.tensor_mul(out=msgs2[:, k % MRING, :], in0=ps_f2[:], in1=xs[:])

        def scatter_chunk(c):
            s_sb = spool.tile([128, WSTAT, 128], f32r, tag="s_oh")
            nc.sync.dma_start(out=s_sb[:], in_=s_d[c])
            if c % 4 == 0:
                agg_cur[0] = stream.tile([128, 2, 512], f32r, tag="agg1", name="agg1t")
                agg_cur[1] = stream.tile([128, 2, 512], f32r, tag="agg2", name="agg2t")
            for br, (msgs, ptag, atag) in enumerate(
                    ((msgs1, "pC", "pC"), (msgs2, "pD", "pD"))):
                ps_a = psumd.tile([128, H], f32, tag=ptag)
                for w in range(WSTAT):
                    kk = kstart[c] + w
                    nc.tensor.matmul(ps_a[:], lhsT=s_sb[:, w, :],
                                     rhs=msgs[:, kk % MRING, :],
                                     start=(w == 0), stop=(w == WSTAT - 1))
                agg_nm = stream.tile([128, H], f32, tag="aggnm")
                nc.vector.tensor_copy(out=agg_nm[:], in_=ps_a[:])
                for hc in range(2):
                    ps_t = psumd.tile([128, 128], f32, tag=atag)
                    nc.tensor.transpose(ps_t[:], agg_nm[:, hc * 128:(hc + 1) * 128], ident[:])
                    nc.vector.tensor_copy(
                        out=agg_cur[br][:, hc, (c % 4) * 128:(c % 4 + 1) * 128],
                        in_=ps_t[:])

        def conv_group(n4):
            nsl = slice(n4 * 512, (n4 + 1) * 512)
            for br in range(2):
                aggX = agg_cur[br]
                if br == 0:
                    cllt, clrt, clb, lint, linb_b = c1llt, c1lrt, c1llb_pp, lin1t, lin1b_pp
                else:
                    cllt, clrt, clb, lint, linb_b = c2llt, c2lrt, c2llb_pp, lin2t, lin2b_pp
                inner = s3.tile([128, 2, 512], f32r, tag="n2x512")
                for ho in range(2):
                    hsl = slice(ho * 128, (ho + 1) * 128)
                    ps = psum.tile([128, 512], f32, tag="pE")
                    nc.tensor.matmul(ps[:], lhsT=cllt[:, 0, hsl], rhs=aggX[:, 0, :],
                                     start=True, stop=False)
                    nc.tensor.matmul(ps[:], lhsT=cllt[:, 1, hsl], rhs=aggX[:, 1, :],
                                     start=False, stop=False)
                    nc.tensor.matmul(ps[:], lhsT=clrt[:, 0, hsl], rhs=xlocT[:, 0, nsl],
                                     start=False, stop=False)
                    nc.tensor.matmul(ps[:], lhsT=clrt[:, 1, hsl], rhs=xlocT[:, 1, nsl],
                                     start=False, stop=True)
                    nc.scalar.activation(out=inner[:, ho, :], in_=ps[:], func=AF.Identity,
                                         bias=clb[:, ho:ho + 1])
                hb = s3.tile([128, 2, 512], f32r, tag="n2x512")
                for ho in range(2):
                    hsl = slice(ho * 128, (ho + 1) * 128)
                    ps2 = psum.tile([128, 512], f32, tag="pE")
                    for hc in range(2):
                        nc.tensor.matmul(ps2[:], lhsT=lint[:, hc, hsl],
                                         rhs=inner[:, hc, :],
                                         start=(hc == 0), stop=(hc == 1))
                    silu_act(hb[:, ho, :], ps2[:], linb_b[:, ho:ho + 1])
                for ho in range(2):
                    hsl = slice(ho * 128, (ho + 1) * 128)
                    ps3 = psum.tile([128, 512], f32, tag="pE")
                    for hc in range(2):
                        nc.tensor.matmul(ps3[:], lhsT=lincatt[:, br * 2 + hc, hsl],
                                         rhs=hb[:, hc, :],
                                         start=(hc == 0), stop=(hc == 1))
                    if br == 0:
                        nc.vector.tensor_copy(out=hcat[:, ho, nsl], in_=ps3[:])
                    else:
                        tmp = stream.tile([128, 512], f32, tag="tmp512")
                        nc.vector.tensor_add(out=tmp[:], in0=ps3[:], in1=hcat[:, ho, nsl])
                        nc.scalar.activation(out=tmp[:], in_=tmp[:], func=AF.Identity,
                                             bias=lincatb_pp[:, ho:ho + 1])
                        nc.vector.tensor_add(out=hT[:, ho, nsl], in0=tmp[:],
                                             in1=xlocT[:, ho, nsl])

        for k in range(NKC):
            produce_chunk(k)
            for c in trigger[k]:
                scatter_chunk(c)
                if c % 4 == 3:
                    conv_group(c // 4)

        # ---- residual lins (in place on hT; both ho psums read before writes) ----
        for l in range(3):
            for n4 in range(NCAP // 512):
                nsl = slice(n4 * 512, (n4 + 1) * 512)
                pss = []
                for ho in range(2):
                    hsl = slice(ho * 128, (ho + 1) * 128)
                    ps = psumd.tile([128, 512], f32, tag="pB")
                    for hc in range(2):
                        nc.tensor.matmul(ps[:], lhsT=linst[:, l * 2 + hc, hsl],
                                         rhs=hT[:, hc, nsl],
                                         start=(hc == 0), stop=(hc == 1))
                    pss.append(ps)
                for ho in range(2):
                    sw = stream.tile([128, 512], f32, tag="tmp512")
                    silu_act(sw[:], pss[ho][:], linsb_pp[:, l * 2 + ho:l * 2 + ho + 1])
                    nc.vector.tensor_add(out=hT[:, ho, nsl], in0=sw[:], in1=hT[:, ho, nsl])

        # ---- GraphNorm ----
        h_nm = big.tile([128, NNC, H], f32r, tag="xlocT")
        for c in range(NNC):
            for hc in range(2):
                ps_t = psumd.tile([128, 128], f32, tag="pC")
                nc.tensor.transpose(ps_t[:], hT[:, hc, c * 128:(c + 1) * 128].bitcast(f32),
                                    ident[:])
                nc.vector.tensor_copy(out=h_nm[:, c, hc * 128:(hc + 1) * 128], in_=ps_t[:])
        sq_nm = big.tile([128, NNC, H], f32r, tag="hcat")
        nc.vector.tensor_mul(out=sq_nm[:], in0=h_nm[:], in1=h_nm[:])

        ps_sh = psum.tile([NGC, H], f32, tag="pA")
        ps_sq = psumd.tile([NGC, H], f32, tag="pB")
        for c in range(NNC):
            nc.tensor.matmul(ps_sh[:], lhsT=g_oh[:, c, :], rhs=h_nm[:, c, :],
                             start=(c == 0), stop=(c == NNC - 1))
            nc.tensor.matmul(ps_sq[:], lhsT=g_oh[:, c, :], rhs=sq_nm[:, c, :],
                             start=(c == 0), stop=(c == NNC - 1))
        cnt = const.tile([NGC, 1], f32)
        nc.vector.tensor_reduce(cnt[:], gt_oh[:].bitcast(f32), axis=mybir.AxisListType.X,
                                op=mybir.AluOpType.add)
        inv_cnt = const.tile([NGC, 1], f32)
        nc.vector.tensor_scalar_max(inv_cnt[:], cnt[:], 1.0)
        nc.vector.reciprocal(out=inv_cnt[:], in_=inv_cnt[:])
        mean = const.tile([NGC, H], f32)
        nc.vector.tensor_tensor(out=mean[:], in0=ps_sh[:],
                                in1=inv_cnt[:].to_broadcast((NGC, H)),
                                op=mybir.AluOpType.mult)
        meansq = const.tile([NGC, H], f32)
        nc.vector.tensor_tensor(out=meansq[:], in0=ps_sq[:],
                                in1=inv_cnt[:].to_broadcast((NGC, H)),
                                op=mybir.AluOpType.mult)
        am = const.tile([NGC, H], f32r)
        nc.vector.tensor_mul(out=am[:], in0=alpha16[:], in1=mean[:])
        t2m = const.tile([NGC, H], f32)
        nc.vector.tensor_scalar_mul(t2m[:], mean[:], 2.0)
        nc.vector.tensor_sub(out=t2m[:], in0=t2m[:], in1=am[:].bitcast(f32))
        nc.vector.tensor_mul(out=t2m[:], in0=am[:].bitcast(f32), in1=t2m[:])
        var = const.tile([NGC, H], f32)
        nc.vector.tensor_sub(out=var[:], in0=meansq[:], in1=t2m[:])
        nc.vector.tensor_scalar_add(var[:], var[:], float(EPS))
        std = const.tile([NGC, H], f32)
        nc.scalar.activation(out=std[:], in_=var[:], func=AF.Sqrt)
        rstd32 = const.tile([NGC, H], f32)
        nc.vector.reciprocal(out=rstd32[:], in_=std[:])
        rstd = const.tile([NGC, H], f32r)
        nc.vector.tensor_copy(out=rstd[:], in_=rstd32[:])

        for n4 in range(NCAP // 512):
            nsl = slice(n4 * 512, (n4 + 1) * 512)
            for ho in range(2):
                hsl = slice(ho * 128, (ho + 1) * 128)
                ps_am = psumd.tile([128, 512], f32, tag="pC")
                nc.tensor.matmul(ps_am[:], lhsT=am[:, hsl], rhs=gt_oh[:, nsl],
                                 start=True, stop=True)
                ps_rs = psumd.tile([128, 512], f32, tag="pD")
                nc.tensor.matmul(ps_rs[:], lhsT=rstd[:, hsl], rhs=gt_oh[:, nsl],
                                 start=True, stop=True)
                t = stream.tile([128, 512], f32, tag="tmp512")
                nc.vector.tensor_sub(out=t[:], in0=hT[:, ho, nsl], in1=ps_am[:])
                nc.vector.tensor_mul(out=t[:], in0=t[:], in1=ps_rs[:])
                nc.scalar.activation(out=hT[:, ho, nsl], in_=t[:], func=AF.Identity,
                                     scale=gamma_pp[:, ho:ho + 1],
                                     bias=beta_pp[:, ho:ho + 1])

        # ---- final linear ----
        outt_r = outt_d[:].rearrange("(a p) n -> p a n", p=128)
        for n4 in range(NCAP // 512):
            nsl = slice(n4 * 512, (n4 + 1) * 512)
            for ho in range(2):
                hsl = slice(ho * 128, (ho + 1) * 128)
                ps = psumd.tile([128, 512], f32, tag="pB")
                for hc in range(2):
                    nc.tensor.matmul(ps[:], lhsT=finalt[:, hc, hsl],
                                     rhs=hT[:, hc, nsl],
                                     start=(hc == 0), stop=(hc == 1))
                ot = stream.tile([128, 512], f32, tag="tmp512")
                nc.scalar.activation(out=ot[:], in_=ps[:], func=AF.Identity,
                                     bias=finalb_pp[:, ho:ho + 1])
                nc.sync.dma_start(out=outt_r[:, ho, nsl], in_=ot[:])

    nc.compile()
    return nc


def _get_program(sim_compat=False):
    key = ("sim" if sim_compat else "hw")
    if key not in _PROG_CACHE:
        _PROG_CACHE[key] = _build_program(sim_compat)
    return _PROG_CACHE[key]


# ======================================================================
# Host-side sharding
# ======================================================================

def _pp(b):  # [256] -> per-partition [128, 2] (ho-chunk columns)
    return np.ascontiguousarray(b.reshape(2, 128).T, dtype=np.float32)


def _shared_weights(inp):
    f32 = np.float32
    w = {}
    w["w1"] = np.ascontiguousarray(inp["f1_w1"], f32)
    w["w2t1"] = np.ascontiguousarray(inp["f1_w2"].T, f32)
    w["w12"] = np.ascontiguousarray(inp["f2_w1"], f32)
    w["w2t2"] = np.ascontiguousarray(inp["f2_w2"].T, f32)
    for name, key in [("linwt", "lin_w"), ("c1llt", "c1_ll_w"), ("c1lrt", "c1_lr_w"),
                      ("c2llt", "c2_ll_w"), ("c2lrt", "c2_lr_w"),
                      ("lin1t", "lin1_w"), ("lin2t", "lin2_w"), ("finalt", "final_w")]:
        w[name] = np.ascontiguousarray(np.asarray(inp[key], f32).T)
    w["lincatt"] = np.ascontiguousarray(np.asarray(inp["lincat_w"], f32).T)  # [512,256]
    w["linst"] = np.ascontiguousarray(
        np.concatenate([np.asarray(inp["lins_w"][l], f32).T for l in range(3)], axis=0))
    w["linb_row"] = np.asarray(inp["lin_b"], f32).reshape(1, H).copy()
    w["linb_pp"] = _pp(np.asarray(inp["lin_b"], f32))
    w["c1llb_pp"] = _pp(np.asarray(inp["c1_ll_b"], f32))
    w["c2llb_pp"] = _pp(np.asarray(inp["c2_ll_b"], f32))
    w["lin1b_pp"] = _pp(np.asarray(inp["lin1_b"], f32))
    w["lin2b_pp"] = _pp(np.asarray(inp["lin2_b"], f32))
    w["lincatb_pp"] = _pp(np.asarray(inp["lincat_b"], f32))
    w["linsb_pp"] = np.concatenate(
        [_pp(np.asarray(inp["lins_b"][l], f32)) for l in range(3)], axis=1)  # [128, 6]
    w["finalb_pp"] = _pp(np.asarray(inp["final_b"], f32))
    w["gamma_pp"] = _pp(np.asarray(inp["norm_gamma"], f32))
    w["beta_pp"] = _pp(np.asarray(inp["norm_beta"], f32))
    w["alpha_row"] = np.asarray(inp["norm_alpha"], f32).reshape(1, H).copy()
    return w


def _shard(inp):
    f32 = np.float32
    x = np.asarray(inp["x"], f32)
    f1 = np.asarray(inp["feature1"], f32)
    f2 = np.asarray(inp["feature2"], f32)
    ei = np.asarray(inp["edge_index"]).astype(np.int64)
    batch = np.asarray(inp["batch"]).astype(np.int64)
    src, tgt = ei[0], ei[1]

    gn_counts = np.bincount(batch, minlength=NG)          # nodes per graph
    ge_counts = np.bincount(batch[tgt], minlength=NG)     # edges per graph (by target)
    gn_start = np.concatenate([[0], np.cumsum(gn_counts)])

    # contiguous graph partition balancing edges
    cume = np.cumsum(ge_counts)
    bounds = [0]
    for c in range(1, NCORES):
        target = cume[-1] * c / NCORES
        g = int(np.searchsorted(cume, target))
        bounds.append(max(bounds[-1] + 1, min(g + 1, NG - (NCORES - c))))
    bounds.append(NG)

    sdt = np.dtype("bfloat16") if S_BF16 else f32
    if S_BF16:
        import ml_dtypes
        sdt = ml_dtypes.bfloat16

    w = _shared_weights(inp)
    in_maps = []
    meta = []
    WIN = W if DYNAMIC_WINDOWS else WSTAT
    for c in range(NCORES):
        glo, ghi = bounds[c], bounds[c + 1]
        ns, ne = int(gn_start[glo]), int(gn_start[ghi])
        ncnt = ne - ns
        assert ncnt <= NCAP, f"core {c}: {ncnt} nodes > NCAP"
        assert ghi - glo <= NGC, f"core {c}: {ghi - glo} graphs > NGC"

        emask = (tgt >= ns) & (tgt < ne)
        eidx = np.nonzero(emask)[0]
        loc_t = tgt[eidx] - ns
        order = np.argsort(loc_t, kind="stable")
        eidx = eidx[order]
        loc_t = loc_t[order]
        ecnt = len(eidx)
        assert ecnt <= ECAP, f"core {c}: {ecnt} edges > ECAP"

        f1_sh = np.zeros((ECAP, F1), f32)
        f1_sh[:ecnt] = f1[eidx]
        # [NKC, KP, KF1*128]: partition p holds K-chunk rows (o*KP+p) contiguous
        f1t = np.ascontiguousarray(
            f1_sh.reshape(NKC, 128, KF1, KP).transpose(0, 3, 2, 1).reshape(NKC, KP, KF1 * 128))
        f2_sh = np.zeros((ECAP, F2), f32)
        f2_sh[:ecnt] = f2[eidx]
        f2t = np.ascontiguousarray(
            f2_sh.reshape(NKC, 128, KF2, KP).transpose(0, 3, 2, 1).reshape(NKC, KP, KF2 * 128))
        xs_sh = np.zeros((ECAP, H), f32)
        xs_sh[:ecnt] = x[src[eidx]]
        xsrct = np.ascontiguousarray(
            xs_sh.reshape(NKC, 128, 2, 128).transpose(0, 3, 2, 1).reshape(NKC, 128, 2 * 128))
        xloc = np.zeros((NCAP, H), f32)
        xloc[:ncnt] = x[ns:ne]
        xloct = np.ascontiguousarray(xloc.T)

        cum = np.searchsorted(loc_t, np.arange(NNC) * 128)
        if DYNAMIC_WINDOWS:
            kstart = np.clip(cum // 128, 0, NKC - W)
        else:
            kstart = np.clip(4 * np.arange(NNC) - (WSTAT - 5) // 2, 0, NKC - WSTAT)
        slots = np.arange(ecnt)
        kk = slots // 128
        cc = loc_t // 128
        ww = kk - kstart[cc]
        assert (ww >= 0).all() and (ww < WIN).all(), f"core {c}: window overflow"
        s_oh = np.zeros((NNC, 128, WIN, 128), sdt)
        s_oh[cc, slots % 128, ww, loc_t - cc * 128] = 1
        widx = (kstart * H).astype(np.int32).reshape(1, NNC)

        g_loc = batch[ns:ne] - glo
        nl = np.arange(ncnt)
        g_oh = np.zeros((NNC, 128, NGC), f32)
        g_oh[nl // 128, nl % 128, g_loc] = 1
        gt_oh = np.zeros((NGC, NCAP), f32)
        gt_oh[g_loc, nl] = 1

        m = {"f1t": f1t, "f2t": f2t, "xsrct": xsrct, "xloct": xloct,
             "s_oh": s_oh, "widx": widx, "g_oh": g_oh, "gt_oh": gt_oh,
             "ones": np.ones((1, 128), f32)}
        m.update(w)
        in_maps.append(m)
        meta.append((ns, ne))
    return in_maps, meta


def kernel(**inputs):
    from concourse.bass_utils import run_bass_kernel_spmd

    nc = _get_program()
    in_maps, meta = _shard(inputs)
    res = run_bass_kernel_spmd(nc, in_maps, list(range(NCORES)))
    out = np.empty((N, H), np.float32)
    for c, (ns, ne) in enumerate(meta):
        out[ns:ne] = res.results[c]["outt"][:, :ne - ns].T
    return out
